# revision 1
# baseline (speedup 1.0000x reference)
"""Multi-head attention kernel for 8 Trainium2 NeuronCores.

Problem: B=4, S=2048, D=1024, H=16, Dh=64 MHA with key-side boolean mask.

Sharding: core c handles (batch b = c//2, head-half g = c%2, 8 heads each).
QKV are column-parallel, the output projection is row-parallel (Megatron
style); the host sums the two partial output projections per batch and adds
the output bias.

Host-side preprocessing (pure data marshalling, exact):
  - All inputs are pre-tiled into DMA-native layouts (partition-major,
    contiguous per partition) so each dma_start lowers to large linear
    descriptors instead of thousands of 2KB strided reads.
  - x is transposed per batch (the PE contracts over the partition dim, so
    x^T is required for every projection).
  - Keys with mask=False contribute exactly zero after softmax, so the host
    gathers only the unmasked keys (padded to a multiple of 384 with zero
    rows whose exp-bias is -1e30 => exp == 0 exactly). Exact, and cuts
    score/exp/attn-V work roughly in half.

On-core dataflow (all matmuls in float32r):
  xT --(Wk,Wv)--> KT[f,k] (zero-padded per head to K=128), V[k,f] (+biases)
  xT --(Wq)--> QT[f,q]
  scores^T[k,q] = [KT_h ; 0]^T x QT_pair   (K=128 full array; the zero rows
                                            kill the other head's features)
  E = exp(scores*0.125 + maskbias[k])      (one ScalarE pass, mask fused)
  out_aug[65,q] = [V_h | ones]^T x E       (row 64 = softmax denominator)
  attnT[f,q] = out_aug[0:64] * bcast(1/den)  (K=1 ones matmul broadcast)
  out[s,D] = attnT^T x Wo                  (partial; host adds pair + bo)
"""

import os
import numpy as np

os.environ.setdefault("MYCRO_LOCAL_CACHE", "1")

D_MODEL = 1024
N_HEADS = 16
D_HEAD = 64
BATCH = 4
SEQ = 2048
N_CORES = 8
FH = 512          # features per core (8 heads x 64)
HPC = 8           # heads per core
NEG = -1.0e30     # additive bias for padded/masked keys; exp -> 0 exactly

_COMPILED = {}    # k_pad -> nc
last_results = None  # BassKernelResults of the most recent run (for test.py)


def _build(k_pad):
    """Emit + compile the per-core bass kernel for a given padded key count."""
    import concourse.bacc as bacc
    import concourse.bass as bass
    import concourse.tile as tile
    from concourse import mybir

    f32 = mybir.dt.float32
    f32r = mybir.dt.float32r
    KT_N = k_pad // 128                     # number of 128-key tiles
    KC = 512 if k_pad % 512 == 0 else 384   # key-side chunk (fp32r needs N>=256)
    assert k_pad % KC == 0 and KC % 128 == 0
    NKC = k_pad // KC

    nc = bacc.Bacc("TRN2", target_bir_lowering=False, debug=False,
                   num_devices=N_CORES)

    # all pre-tiled on host into DMA-native layouts
    dxq = nc.dram_tensor("xq", [4, 128, 8, 512], f32r, kind="ExternalInput")
    dxk = nc.dram_tensor("xk", [NKC, 128, 8, KC], f32r, kind="ExternalInput")
    dWq = nc.dram_tensor("Wq", [128, 8, FH], f32r, kind="ExternalInput")
    dWk = nc.dram_tensor("Wk", [128, 8, FH], f32r, kind="ExternalInput")
    dWv = nc.dram_tensor("Wv", [128, 8, HPC * 65], f32r, kind="ExternalInput")
    dWo = nc.dram_tensor("Wo", [128, 4, D_MODEL], f32r, kind="ExternalInput")
    dbc = nc.dram_tensor("bcst", [128, 8 + KT_N], f32, kind="ExternalInput")
    dbv = nc.dram_tensor("bv", [HPC * 65], f32r, kind="ExternalInput")
    dcst = nc.dram_tensor("consts", [256], f32r, kind="ExternalInput")  # ones|zeros
    dzp = nc.dram_tensor("zpad", [4 * k_pad], f32r, kind="ExternalInput")
    dout = nc.dram_tensor("out", [SEQ, D_MODEL], f32, kind="ExternalOutput")

    EXP = mybir.ActivationFunctionType.Exp
    IDn = mybir.ActivationFunctionType.Identity

    with tile.TileContext(nc) as tc:
        with tc.tile_pool(name="persist", bufs=1) as pers:
            # ---- constants in SBUF ----
            bc = pers.tile([128, 8 + KT_N], f32, tag="bcst")
            nc.sync.dma_start(out=bc, in_=dbc.ap())
            bq = bc[:, 0:4]
            bk = bc[:, 4:8]
            mb = bc[:, 8:8 + KT_N]
            bv_row = pers.tile([1, HPC * 65], f32r, tag="bvr")
            nc.sync.dma_start(out=bv_row, in_=dbv.ap()[None, :])
            ones_t = pers.tile([1, 128], f32r, tag="ones")
            nc.sync.dma_start(out=ones_t, in_=dcst.ap()[None, 0:128])
            ones64 = ones_t[:, 0:64]
            ones128 = ones_t[:, :]

            # ---- persistent activations ----
            QT = pers.tile([128, 4, SEQ], f32r, tag="QT")        # [f, q]
            # zero-padded per-head score weights: KTe rows 0:64 = even head,
            # rows 64:128 = 0; KTo rows 0:64 = 0, rows 64:128 = odd head.
            KTe = pers.tile([128, 4, k_pad], f32r, tag="KTe")
            KTo = pers.tile([128, 4, k_pad], f32r, tag="KTo")
            Vau = pers.tile([128, KT_N, HPC, 65], f32r, tag="Vau")

            zin = bass.AP(tensor=dzp.ap().tensor, offset=0,
                          ap=[[0, 64], [1, 4 * k_pad]])
            nc.sync.dma_start(
                out=KTo[0:64, :, :].rearrange("p a k -> p (a k)"), in_=zin)

            # ================= projections =================
            wtq_cm = tc.tile_pool(name="wtq", bufs=1)
            wtq = wtq_cm.__enter__()
            wq = wtq.tile([128, 8, FH], f32r, tag="wq")
            nc.sync.dma_start(out=wq, in_=dWq.ap())
            ppool_cm = tc.tile_pool(name="pp", bufs=4, space="PSUM")
            ppool = ppool_cm.__enter__()

            # ----- K side (KT, V) -----
            with tc.tile_pool(name="wtk", bufs=1) as wtk, \
                 tc.tile_pool(name="xk", bufs=2) as xkp:
                pk = ppool
                wk = wtk.tile([128, 8, FH], f32r, tag="wk")
                nc.sync.dma_start(out=wk, in_=dWk.ap())
                wv = wtk.tile([128, 8, HPC * 65], f32r, tag="wv")
                nc.sync.dma_start(out=wv, in_=dWv.ap())
                for kc in range(NKC):
                    xk_t = xkp.tile([128, 8, KC], f32r, tag="xk")
                    nc.sync.dma_start(out=xk_t, in_=dxk.ap()[kc])
                    for ft in range(4):
                        ps = pk.tile([128, KC], f32, tag="pk")
                        for dt in range(8):
                            nc.tensor.matmul(
                                ps,
                                lhsT=wk[:, dt, ft * 128:(ft + 1) * 128],
                                rhs=xk_t[:, dt, :],
                                start=(dt == 0), stop=(dt == 7))
                        ks = slice(kc * KC, (kc + 1) * KC)
                        nc.scalar.activation(KTe[:, ft, ks], ps, IDn,
                                             bias=bk[:, ft:ft + 1])
                    for kb in range(KC // 128):
                        kg = kc * (KC // 128) + kb
                        ps = pk.tile([128, HPC * 65], f32, tag="pk")
                        for dt in range(8):
                            nc.tensor.matmul(
                                ps[:, 0:512],
                                lhsT=xk_t[:, dt, kb * 128:(kb + 1) * 128],
                                rhs=wv[:, dt, 0:512],
                                start=(dt == 0), stop=False)
                            nc.tensor.matmul(
                                ps[:, 512:520],
                                lhsT=xk_t[:, dt, kb * 128:(kb + 1) * 128],
                                rhs=wv[:, dt, 512:520],
                                start=(dt == 0), stop=False)
                        nc.tensor.matmul(ps[:, 0:512], lhsT=ones128,
                                         rhs=bv_row[:, 0:512],
                                         start=False, stop=True)
                        nc.tensor.matmul(ps[:, 512:520], lhsT=ones128,
                                         rhs=bv_row[:, 512:520],
                                         start=False, stop=True)
                        nc.scalar.copy(Vau[:, kg, :, :], ps)

            # finish zero-padded score weights: KTo upper half is a copy of
            # the drained KTe upper half; then KTe upper half becomes zero.
            nc.vector.tensor_copy(
                KTo[64:128, :, :].rearrange("p a k -> p (a k)"),
                KTe[64:128, :, :].rearrange("p a k -> p (a k)"))
            nc.sync.dma_start(
                out=KTe[64:128, :, :].rearrange("p a k -> p (a k)"), in_=zin)

            # ----- Q side (QT) -----
            with tc.tile_pool(name="xq", bufs=2) as xqp:
                pq = ppool
                for qc in range(4):
                    xq_t = xqp.tile([128, 8, 512], f32r, tag="xq")
                    nc.sync.dma_start(out=xq_t, in_=dxq.ap()[qc])
                    for ft in range(4):
                        ps = pq.tile([128, 512], f32, tag="pk")
                        for dt in range(8):
                            nc.tensor.matmul(
                                ps,
                                lhsT=wq[:, dt, ft * 128:(ft + 1) * 128],
                                rhs=xq_t[:, dt, :],
                                start=(dt == 0), stop=(dt == 7))
                        nc.scalar.activation(QT[:, ft, qc * 512:(qc + 1) * 512],
                                             ps, IDn, bias=bq[:, ft:ft + 1])

            ppool_cm.__exit__(None, None, None)
            wtq_cm.__exit__(None, None, None)

            # ================= attention core =================
            att2_cm = tc.tile_pool(name="att2", bufs=1)
            att2 = att2_cm.__enter__()
            attnT = att2.tile([128, 4, SEQ], f32r, tag="attnT")  # [f, q]
            wo = att2.tile([128, 4, D_MODEL], f32r, tag="wo")
            nc.sync.dma_start(out=wo, in_=dWo.ap())
            with tc.tile_pool(name="et", bufs=4) as etp, \
                 tc.tile_pool(name="ua", bufs=2) as uap, \
                 tc.tile_pool(name="rp", bufs=2) as rpp, \
                 tc.tile_pool(name="sp", bufs=2, space="PSUM") as sp, \
                 tc.tile_pool(name="av", bufs=2, space="PSUM") as avp:
                for t in range(4):          # head pair (heads 2t, 2t+1)
                    for qh in range(2):     # query half (1024 queries)
                        q0 = qh * 1024
                        avA = avp.tile([65, 1024], f32, tag="av")
                        avB = avp.tile([65, 1024], f32, tag="av")
                        for kt in range(KT_N):
                            kts = slice(kt * 128, (kt + 1) * 128)
                            sA = sp.tile([128, 1024], f32, tag="s")
                            sB = sp.tile([128, 1024], f32, tag="s")
                            for h in range(2):
                                c0, c1 = q0 + h * 512, q0 + (h + 1) * 512
                                nc.tensor.matmul(
                                    sA[:, h * 512:(h + 1) * 512],
                                    lhsT=KTe[:, t, kts], rhs=QT[:, t, c0:c1],
                                    start=True, stop=True)
                                nc.tensor.matmul(
                                    sB[:, h * 512:(h + 1) * 512],
                                    lhsT=KTo[:, t, kts], rhs=QT[:, t, c0:c1],
                                    start=True, stop=True)
                            eA = etp.tile([128, 1024], f32r, tag="et")
                            nc.scalar.activation(eA, sA, EXP,
                                                 bias=mb[:, kt:kt + 1], scale=0.125)
                            eB = etp.tile([128, 1024], f32r, tag="et")
                            nc.scalar.activation(eB, sB, EXP,
                                                 bias=mb[:, kt:kt + 1], scale=0.125)
                            for h in range(2):
                                cs = slice(h * 512, (h + 1) * 512)
                                nc.tensor.matmul(
                                    avA[:, cs], lhsT=Vau[:, kt, 2 * t, :],
                                    rhs=eA[:, cs],
                                    start=(kt == 0), stop=(kt == KT_N - 1))
                                nc.tensor.matmul(
                                    avB[:, cs], lhsT=Vau[:, kt, 2 * t + 1, :],
                                    rhs=eB[:, cs],
                                    start=(kt == 0), stop=(kt == KT_N - 1))
                        # normalize: attnT = out_aug[0:64] * bcast(1/den)
                        rA = rpp.tile([1, 1024], f32r, tag="r")
                        rB = rpp.tile([1, 1024], f32r, tag="r")
                        with nc.allow_low_precision(reason="fp32r matmul operand"):
                            nc.vector.reciprocal(rA, avA[64:65, :])
                            nc.vector.reciprocal(rB, avB[64:65, :])
                        bcA = sp.tile([64, 1024], f32, tag="s")
                        bcB = sp.tile([64, 1024], f32, tag="s")
                        for h in range(2):
                            cs = slice(h * 512, (h + 1) * 512)
                            nc.tensor.matmul(bcA[:, cs], lhsT=ones64,
                                             rhs=rA[:, cs], start=True, stop=True)
                            nc.tensor.matmul(bcB[:, cs], lhsT=ones64,
                                             rhs=rB[:, cs], start=True, stop=True)
                        # DVE reads at most one PSUM operand: stage out_aug's
                        # attn rows through SBUF, then multiply.
                        uA = uap.tile([64, 1024], f32, tag="ua")
                        nc.scalar.copy(uA, avA[0:64, :])
                        uB = uap.tile([64, 1024], f32, tag="ua")
                        nc.scalar.copy(uB, avB[0:64, :])
                        nc.vector.tensor_mul(attnT[0:64, t, q0:q0 + 1024],
                                             uA, bcA)
                        nc.vector.tensor_mul(attnT[64:128, t, q0:q0 + 1024],
                                             uB, bcB)

            # ================= output projection (partial) =================
            with tc.tile_pool(name="op", bufs=2, space="PSUM") as opp, \
                 tc.tile_pool(name="ot", bufs=3) as otp:
                for st in range(16):
                    ps = opp.tile([128, D_MODEL], f32, tag="op")
                    for ft in range(4):
                        for dh in range(2):
                            nc.tensor.matmul(
                                ps[:, dh * 512:(dh + 1) * 512],
                                lhsT=attnT[:, ft, st * 128:(st + 1) * 128],
                                rhs=wo[:, ft, dh * 512:(dh + 1) * 512],
                                start=(ft == 0), stop=(ft == 3))
                    ot = otp.tile([128, D_MODEL], f32, tag="ot")
                    nc.scalar.copy(ot, ps)
                    nc.sync.dma_start(out=dout.ap()[st * 128:(st + 1) * 128, :], in_=ot)
            att2_cm.__exit__(None, None, None)

    nc.compile()
    return nc


def _get_compiled(k_pad):
    if k_pad not in _COMPILED:
        _COMPILED[k_pad] = _build(k_pad)
    return _COMPILED[k_pad]


def _tile_pf(a, p=128):
    """[P*t, f...] -> contiguous [p, t, f...] partition-major tiling."""
    t = a.shape[0] // p
    return np.ascontiguousarray(
        a.reshape(t, p, *a.shape[1:]).swapaxes(0, 1))


def _prep_core_inputs(x, attention_mask, Wq, bq, Wk, bk, Wv, bv, Wo):
    """Host-side shard prep. Returns (in_maps, k_pad)."""
    x = np.asarray(x, np.float32)
    mask = np.asarray(attention_mask, bool)
    idxs = [np.nonzero(mask[b])[0] for b in range(BATCH)]
    ke_max = max(1, max(len(i) for i in idxs))
    k_pad = 384 * ((ke_max + 383) // 384)
    if k_pad > SEQ:
        k_pad = SEQ
    KC = 512 if k_pad % 512 == 0 else 384
    NKC = k_pad // KC
    KT_N = k_pad // 128

    consts = np.zeros(256, np.float32)
    consts[0:128] = 1.0

    in_maps = []
    for b in range(BATCH):
        xT = x[b].T                                  # [D, S] view
        # xq: [qc, p, dt, 512]
        xq = np.ascontiguousarray(
            xT.reshape(8, 128, 4, 512).transpose(2, 1, 0, 3))
        idx = idxs[b]
        ke = len(idx)
        if ke > k_pad:
            idx = idx[:k_pad]
            ke = k_pad
        xkT = np.zeros((D_MODEL, k_pad), np.float32)
        xkT[:, :ke] = x[b][idx].T
        # xk: [kc, p, dt, KC]
        xk = np.ascontiguousarray(
            xkT.reshape(8, 128, NKC, KC).transpose(2, 1, 0, 3))
        maskb = np.zeros(k_pad, np.float32)
        maskb[ke:] = NEG
        mb_t = _tile_pf(maskb)                       # [128, KT_N]
        KT_N = k_pad // 128
        for g in range(2):
            fs = slice(g * FH, (g + 1) * FH)
            # Wv/bv padded with a ones column per head: the V-projection
            # matmul then produces [V_h | ones] directly (col = 0*x + 1.0).
            Wv_aug = np.zeros((D_MODEL, HPC * 65), np.float32)
            bv_aug = np.zeros(HPC * 65, np.float32)
            for h in range(HPC):
                Wv_aug[:, h * 65:h * 65 + 64] = Wv[:, g * FH + h * 64:
                                                   g * FH + (h + 1) * 64]
                bv_aug[h * 65:h * 65 + 64] = bv[g * FH + h * 64:
                                                g * FH + (h + 1) * 64]
                bv_aug[h * 65 + 64] = 1.0
            in_maps.append({
                "xq": xq,
                "xk": xk,
                "Wq": _tile_pf(np.asarray(Wq[:, fs], np.float32)),
                "Wk": _tile_pf(np.asarray(Wk[:, fs], np.float32)),
                "Wv": _tile_pf(Wv_aug),
                "Wo": _tile_pf(np.asarray(Wo[fs, :], np.float32)),
                "bcst": np.concatenate(
                    [_tile_pf(np.asarray(bq[fs], np.float32)),
                     _tile_pf(np.asarray(bk[fs], np.float32)),
                     mb_t], axis=1).astype(np.float32),
                "bv": bv_aug,
                "consts": consts,
                "zpad": np.zeros(4 * k_pad, np.float32),
            })
    return in_maps, k_pad


def kernel(x, attention_mask, Wq, bq, Wk, bk, Wv, bv, Wo, bo):
    global last_results
    from concourse.bass_utils import run_bass_kernel_spmd

    in_maps, k_pad = _prep_core_inputs(x, attention_mask, Wq, bq, Wk, bk, Wv, bv, Wo)
    nc = _get_compiled(k_pad)
    res = run_bass_kernel_spmd(nc, in_maps, core_ids=list(range(N_CORES)))
    last_results = res

    bo = np.asarray(bo, np.float32)
    out = np.empty((BATCH, SEQ, D_MODEL), np.float32)
    for b in range(BATCH):
        out[b] = res.results[2 * b]["out"] + res.results[2 * b + 1]["out"] + bo
    return out



# revision 9
# speedup vs baseline: 1.4804x; 1.4804x over previous
"""Multi-head attention kernel for 8 Trainium2 NeuronCores.

Problem: B=4, S=2048, D=1024, H=16, Dh=64 MHA with key-side boolean mask.

Sharding: core c handles (batch b = c//2, head-half g = c%2, 8 heads each).
QKV are column-parallel, the output projection is row-parallel (Megatron
style); the host sums the two partial output projections per batch and adds
the output bias.

Host-side preprocessing (pure data marshalling, exact):
  - All inputs are pre-tiled into DMA-native layouts (partition-major,
    contiguous per partition) so each dma_start lowers to large linear
    descriptors instead of thousands of 2KB strided reads.
  - x is transposed per batch (the PE contracts over the partition dim, so
    x^T is required for every projection).
  - Keys with mask=False contribute exactly zero after softmax, so the host
    gathers only the unmasked keys (padded to a multiple of 384 with zero
    rows whose exp-bias is -1e30 => exp == 0 exactly). Exact, and cuts
    score/exp/attn-V work roughly in half.

On-core dataflow:
  xT --(Wk,Wv)--> KT[f,k] (zero-padded per head to K=128), V[k,f] (+biases)
  xT --(Wq)--> QT[f,q]
  scores^T[k,q] = [KT_h ; 0]^T x QT    (fp32r, K=128 full array; zero rows
                                        kill the other head's features)
  E = exp(scores*0.125 + maskbias[k])  (ScalarE, mask fused, bf16 out)
  av[q,65]  = E_tile^T x [V_h | ones]  (bf16 65-col matmuls; col 64 = den)
  avSB[q,f] = av[:,0:64] * (1/den)     (DVE: recip[128,8] + tensor_scalar,
                                        normalization fused into the
                                        PSUM->SBUF copy, bf16 out)
  attnT[f,q] = PE transpose(avSB)      (bf16 transposes per 128x128 block)
  out[s,D] = attnT^T x Wo              (bf16; partial - host adds pair + bo)
"""

import os
import numpy as np

os.environ.setdefault("MYCRO_LOCAL_CACHE", "1")

D_MODEL = 1024
N_HEADS = 16
D_HEAD = 64
BATCH = 4
SEQ = 2048
N_CORES = 8
FH = 512          # features per core (8 heads x 64)
HPC = 8           # heads per core
NEG = -1.0e30     # additive bias for padded/masked keys; exp -> 0 exactly

_COMPILED = {}    # k_pad -> nc
DEBUG = False
last_results = None  # BassKernelResults of the most recent run (for test.py)


def _build(k_pad):
    """Emit + compile the per-core bass kernel for a given padded key count."""
    import concourse.bacc as bacc
    import concourse.bass as bass
    import concourse.tile as tile
    from concourse import mybir

    f32 = mybir.dt.float32
    f32r = mybir.dt.float32r
    bf16 = mybir.dt.bfloat16
    KT_N = k_pad // 128                     # number of 128-key tiles
    KC = 512 if k_pad % 512 == 0 else 384   # key-side chunk (fp32r needs N>=256)
    assert k_pad % KC == 0 and KC % 128 == 0
    NKC = k_pad // KC

    nc = bacc.Bacc("TRN2", target_bir_lowering=False, debug=False,
                   num_devices=N_CORES)

    # all pre-tiled on host into DMA-native layouts
    dxq = nc.dram_tensor("xq", [4, 128, 8, 512], f32r, kind="ExternalInput")
    dxk = nc.dram_tensor("xk", [NKC, 128, 8, KC], f32r, kind="ExternalInput")
    dWq = nc.dram_tensor("Wq", [128, 8, FH], f32r, kind="ExternalInput")
    dWk = nc.dram_tensor("Wk", [128, 8, FH], f32r, kind="ExternalInput")
    dWv = nc.dram_tensor("Wv", [128, 8, HPC * 65], f32r, kind="ExternalInput")
    dWo = nc.dram_tensor("Wo", [128, 4, D_MODEL], f32r, kind="ExternalInput")
    dbc = nc.dram_tensor("bcst", [128, 8 + KT_N], f32, kind="ExternalInput")
    dbv = nc.dram_tensor("bv", [HPC * 65], f32r, kind="ExternalInput")
    dcst = nc.dram_tensor("consts", [256], f32r, kind="ExternalInput")  # ones|zeros
    dzp = nc.dram_tensor("zpad", [4 * k_pad], f32r, kind="ExternalInput")
    dident = nc.dram_tensor("ident", [128, 128], f32, kind="ExternalInput")
    dout = nc.dram_tensor("out", [SEQ, D_MODEL], f32, kind="ExternalOutput")
    if DEBUG:
        ddq = nc.dram_tensor("dbg_qt", [128, 4, SEQ], f32r, kind="ExternalOutput")
        ddke = nc.dram_tensor("dbg_kte", [128, 4, k_pad], f32r, kind="ExternalOutput")
        ddko = nc.dram_tensor("dbg_kto", [128, 4, k_pad], f32r, kind="ExternalOutput")
        ddv = nc.dram_tensor("dbg_vau", [128, KT_N * HPC * 65], f32, kind="ExternalOutput")
        ddat = nc.dram_tensor("dbg_attnT", [128, 4 * SEQ], f32, kind="ExternalOutput")

    EXP = mybir.ActivationFunctionType.Exp
    ADD = None  # placeholder

    with tile.TileContext(nc) as tc:
        with tc.tile_pool(name="persist", bufs=1) as pers:
            # ---- pools (stack order: wtq > pp > wtk > xk) ----
            wtq_cm = tc.tile_pool(name="wtq", bufs=1)
            wtq = wtq_cm.__enter__()
            ppool_cm = tc.tile_pool(name="pp", bufs=4, space="PSUM")
            ppool = ppool_cm.__enter__()
            wtk_cm = tc.tile_pool(name="wtk", bufs=1)
            wtk = wtk_cm.__enter__()
            xkp_cm = tc.tile_pool(name="xk", bufs=2)
            xkp = xkp_cm.__enter__()

            # ---- DMAs the first matmuls need come first ----
            wk = wtk.tile([128, 8, FH], f32r, tag="wk")
            nc.sync.dma_start(out=wk, in_=dWk.ap())
            wv = wtk.tile([128, 8, HPC * 65], f32r, tag="wv")
            nc.sync.dma_start(out=wv, in_=dWv.ap())
            xk0 = xkp.tile([128, 8, KC], f32r, tag="xk")
            nc.sync.dma_start(out=xk0, in_=dxk.ap()[0])

            # ---- constants in SBUF ----
            bc = pers.tile([128, 8 + KT_N], f32, tag="bcst")
            nc.sync.dma_start(out=bc, in_=dbc.ap())
            bq = bc[:, 0:4]
            bk = bc[:, 4:8]
            mb = bc[:, 8:8 + KT_N]
            bv_row = pers.tile([1, HPC * 65], f32r, tag="bvr")
            nc.sync.dma_start(out=bv_row, in_=dbv.ap()[None, :])
            ones_t = pers.tile([1, 128], f32r, tag="ones")
            nc.sync.dma_start(out=ones_t, in_=dcst.ap()[None, 0:128])
            ones128 = ones_t[:, :]
            ident_f = pers.tile([128, 128], f32, tag="identf")
            nc.sync.dma_start(out=ident_f, in_=dident.ap())
            ident = pers.tile([128, 128], bf16, tag="ident")
            nc.scalar.copy(ident, ident_f)

            # ---- persistent activations ----
            QT = pers.tile([128, 4, SEQ], f32r, tag="QT")        # [f, q]
            # zero-padded per-head score weights: KTe rows 0:64 = even head,
            # rows 64:128 = 0; KTo rows 0:64 = 0, rows 64:128 = odd head.
            KTe = pers.tile([128, 4, k_pad], f32r, tag="KTe")
            KTo = pers.tile([128, 4, k_pad], f32r, tag="KTo")
            Vau = pers.tile([128, KT_N, HPC, 65], bf16, tag="Vau")
            attnT = pers.tile([128, 4, SEQ], bf16, tag="attnT")  # [f, q]
            wo = pers.tile([128, 4, D_MODEL], bf16, tag="wo")

            zin = bass.AP(tensor=dzp.ap().tensor, offset=0,
                          ap=[[0, 64], [1, 4 * k_pad]])
            nc.sync.dma_start(
                out=KTo[0:64, :, :].rearrange("p a k -> p (a k)"), in_=zin)

            wq = wtq.tile([128, 8, FH], f32r, tag="wq")
            nc.sync.dma_start(out=wq, in_=dWq.ap())
            wo_f = wtq.tile([128, 4, D_MODEL], f32r, tag="wof")
            nc.sync.dma_start(out=wo_f, in_=dWo.ap())

            # ================= projections =================
            # ----- K side (KT, V) -----
            pk = ppool
            for kc in range(NKC):
                if kc == 0:
                    xk_t = xk0
                else:
                    xk_t = xkp.tile([128, 8, KC], f32r, tag="xk")
                    nc.sync.dma_start(out=xk_t, in_=dxk.ap()[kc])
                for ft in range(4):
                    ps = pk.tile([128, KC], f32, tag="pk")
                    for dt in range(8):
                        nc.tensor.matmul(
                            ps,
                            lhsT=wk[:, dt, ft * 128:(ft + 1) * 128],
                            rhs=xk_t[:, dt, :],
                            start=(dt == 0), stop=(dt == 7))
                    ks = slice(kc * KC, (kc + 1) * KC)
                    nc.vector.tensor_scalar_add(KTe[:, ft, ks], ps,
                                                bk[:, ft:ft + 1])
                for kb in range(KC // 128):
                    kg = kc * (KC // 128) + kb
                    ps = pk.tile([128, HPC * 65], f32, tag="pk")
                    for dt in range(8):
                        nc.tensor.matmul(
                            ps[:, 0:512],
                            lhsT=xk_t[:, dt, kb * 128:(kb + 1) * 128],
                            rhs=wv[:, dt, 0:512],
                            start=(dt == 0), stop=False)
                        nc.tensor.matmul(
                            ps[:, 512:520],
                            lhsT=xk_t[:, dt, kb * 128:(kb + 1) * 128],
                            rhs=wv[:, dt, 512:520],
                            start=(dt == 0), stop=False)
                    nc.tensor.matmul(ps[:, 0:512], lhsT=ones128,
                                     rhs=bv_row[:, 0:512],
                                     start=False, stop=True)
                    nc.tensor.matmul(ps[:, 512:520], lhsT=ones128,
                                     rhs=bv_row[:, 512:520],
                                     start=False, stop=True)
                    nc.scalar.copy(Vau[:, kg, :, :], ps)

            xkp_cm.__exit__(None, None, None)
            wtk_cm.__exit__(None, None, None)

            # finish zero-padded score weights: KTo upper half is a copy of
            # the drained KTe upper half; then KTe upper half becomes zero.
            nc.vector.tensor_copy(
                KTo[64:128, :, :].rearrange("p a k -> p (a k)"),
                KTe[64:128, :, :].rearrange("p a k -> p (a k)"))
            nc.sync.dma_start(
                out=KTe[64:128, :, :].rearrange("p a k -> p (a k)"), in_=zin)

            # convert Wo to bf16 while the Act engine is idle
            nc.scalar.copy(wo.rearrange("p a d -> p (a d)"),
                           wo_f.rearrange("p a d -> p (a d)"))

            # ----- Q side (QT) -----
            with tc.tile_pool(name="xq", bufs=2) as xqp:
                pq = ppool
                for qc in range(4):
                    xq_t = xqp.tile([128, 8, 512], f32r, tag="xq")
                    nc.sync.dma_start(out=xq_t, in_=dxq.ap()[qc])
                    for ft in range(4):
                        ps = pq.tile([128, 512], f32, tag="pk")
                        for dt in range(8):
                            nc.tensor.matmul(
                                ps,
                                lhsT=wq[:, dt, ft * 128:(ft + 1) * 128],
                                rhs=xq_t[:, dt, :],
                                start=(dt == 0), stop=(dt == 7))
                        nc.vector.tensor_scalar_add(
                            QT[:, ft, qc * 512:(qc + 1) * 512], ps,
                            bq[:, ft:ft + 1])

            ppool_cm.__exit__(None, None, None)
            wtq_cm.__exit__(None, None, None)

            # ================= attention core =================
            # per (t, qh): two F-passes (even/odd head of pair t), each:
            #   software-pipelined kt loop: scores (fp32r) -> exp (bf16)
            #   -> 65-col bf16 attnV matmuls into av[q8][128, 65]
            # tail: recip + tensor_scalar normalize into avSB (bf16),
            # then PE-transpose avSB into attnT.
            with tc.tile_pool(name="et", bufs=3) as etp, \
                 tc.tile_pool(name="asb", bufs=2) as asbp, \
                 tc.tile_pool(name="rp", bufs=4) as rpp, \
                 tc.tile_pool(name="sp", bufs=2, space="PSUM") as sp, \
                 tc.tile_pool(name="avp", bufs=2, space="PSUM") as avp, \
                 tc.tile_pool(name="tpp", bufs=2, space="PSUM") as tpp:
                for t in range(4):          # head pair (heads 2t, 2t+1)
                    for qh in range(2):     # query half (1024 queries)
                        q0 = qh * 1024
                        avSB = asbp.tile([128, 8, 128], bf16, tag="avSB")
                        for F in range(2):  # head 2t+F
                            KT = KTe if F == 0 else KTo
                            h = 2 * t + F
                            # av split in two 1-bank tiles (no accumulation
                            # group may straddle a PSUM bank boundary)
                            av0 = avp.tile([128, 4, 65], f32, tag="av")
                            av1 = avp.tile([128, 4, 65], f32, tag="av")
                            avs = (av0, av1)

                            def scores(kt):
                                s = sp.tile([128, 1024], f32, tag="s")
                                for hh in range(2):
                                    c0 = q0 + hh * 512
                                    nc.tensor.matmul(
                                        s[:, hh * 512:(hh + 1) * 512],
                                        lhsT=KT[:, t, kt * 128:(kt + 1) * 128],
                                        rhs=QT[:, t, c0:c0 + 512],
                                        start=True, stop=True)
                                return s

                            s_cur = scores(0)
                            for kt in range(KT_N):
                                e = etp.tile([128, 1024], bf16, tag="et")
                                nc.scalar.activation(e, s_cur, EXP,
                                                     bias=mb[:, kt:kt + 1],
                                                     scale=0.125)
                                if kt + 1 < KT_N:
                                    s_cur = scores(kt + 1)
                                # PSUM start resets the whole bank: only the
                                # first matmul touching each av bank may set it
                                for q8 in (0, 4, 1, 5, 2, 6, 3, 7):
                                    nc.tensor.matmul(
                                        avs[q8 // 4][:, q8 % 4, :],
                                        lhsT=e[:, q8 * 128:(q8 + 1) * 128],
                                        rhs=Vau[:, kt, h, :],
                                        start=(kt == 0 and q8 % 4 == 0),
                                        stop=(kt == KT_N - 1))
                            # tail: normalize into avSB columns F*64:(F+1)*64
                            r = rpp.tile([128, 8], f32, tag="r")
                            nc.vector.reciprocal(r[:, 0:4], av0[:, :, 64])
                            nc.vector.reciprocal(r[:, 4:8], av1[:, :, 64])
                            for q8 in range(8):
                                nc.vector.tensor_scalar_mul(
                                    avSB[:, q8, F * 64:(F + 1) * 64],
                                    avs[q8 // 4][:, q8 % 4, 0:64],
                                    r[:, q8:q8 + 1])
                        # transpose [q, f] -> [f, q] per 128x128 block
                        for q8 in range(8):
                            tp = tpp.tile([128, 128], bf16, tag="tp")
                            nc.tensor.transpose(tp, avSB[:, q8, :], ident)
                            qs = q0 + q8 * 128
                            nc.vector.tensor_copy(
                                attnT[:, t, qs:qs + 128], tp)

            if DEBUG:
                with tc.tile_pool(name="dbg", bufs=2) as dbp:
                    nc.sync.dma_start(out=ddq.ap(), in_=QT)
                    nc.sync.dma_start(out=ddke.ap(), in_=KTe)
                    nc.sync.dma_start(out=ddko.ap(), in_=KTo)
                    vflat = Vau.rearrange("p a b c -> p (a b c)")
                    for i in range(2):
                        hl = (KT_N * HPC * 65) // 2
                        dv = dbp.tile([128, hl], f32, tag="dv")
                        nc.vector.tensor_copy(dv, vflat[:, i * hl:(i + 1) * hl])
                        nc.sync.dma_start(out=ddv.ap()[:, i * hl:(i + 1) * hl], in_=dv)
                    aflat = attnT.rearrange("p a q -> p (a q)")
                    for i in range(8):
                        da_t = dbp.tile([128, 1024], f32, tag="da")
                        nc.vector.tensor_copy(da_t, aflat[:, i * 1024:(i + 1) * 1024])
                        nc.sync.dma_start(out=ddat.ap()[:, i * 1024:(i + 1) * 1024], in_=da_t)

            # ================= output projection (partial) =================
            with tc.tile_pool(name="op", bufs=2, space="PSUM") as opp, \
                 tc.tile_pool(name="ot", bufs=3) as otp:
                for st in range(16):
                    ps = opp.tile([128, D_MODEL], f32, tag="op")
                    for ft in range(4):
                        for dh in range(2):
                            nc.tensor.matmul(
                                ps[:, dh * 512:(dh + 1) * 512],
                                lhsT=attnT[:, ft, st * 128:(st + 1) * 128],
                                rhs=wo[:, ft, dh * 512:(dh + 1) * 512],
                                start=(ft == 0), stop=(ft == 3))
                    ot = otp.tile([128, D_MODEL], f32, tag="ot")
                    if st % 2 == 0:
                        nc.scalar.copy(ot, ps)
                    else:
                        nc.vector.tensor_copy(ot, ps)
                    nc.sync.dma_start(out=dout.ap()[st * 128:(st + 1) * 128, :], in_=ot)

    nc.compile()
    return nc


def _get_compiled(k_pad):
    key = (k_pad, DEBUG)
    if key not in _COMPILED:
        _COMPILED[key] = _build(k_pad)
    return _COMPILED[key]


def _tile_pf(a, p=128):
    """[P*t, f...] -> contiguous [p, t, f...] partition-major tiling."""
    t = a.shape[0] // p
    return np.ascontiguousarray(
        a.reshape(t, p, *a.shape[1:]).swapaxes(0, 1))


def _prep_core_inputs(x, attention_mask, Wq, bq, Wk, bk, Wv, bv, Wo):
    """Host-side shard prep. Returns (in_maps, k_pad)."""
    x = np.asarray(x, np.float32)
    mask = np.asarray(attention_mask, bool)
    idxs = [np.nonzero(mask[b])[0] for b in range(BATCH)]
    ke_max = max(1, max(len(i) for i in idxs))
    k_pad = 384 * ((ke_max + 383) // 384)
    if k_pad > SEQ:
        k_pad = SEQ
    KC = 512 if k_pad % 512 == 0 else 384
    NKC = k_pad // KC
    KT_N = k_pad // 128

    consts = np.zeros(256, np.float32)
    consts[0:128] = 1.0
    ident = np.eye(128, dtype=np.float32)

    in_maps = []
    for b in range(BATCH):
        xT = x[b].T                                  # [D, S] view
        # xq: [qc, p, dt, 512]
        xq = np.ascontiguousarray(
            xT.reshape(8, 128, 4, 512).transpose(2, 1, 0, 3))
        idx = idxs[b]
        ke = len(idx)
        if ke > k_pad:
            idx = idx[:k_pad]
            ke = k_pad
        xkT = np.zeros((D_MODEL, k_pad), np.float32)
        xkT[:, :ke] = x[b][idx].T
        # xk: [kc, p, dt, KC]
        xk = np.ascontiguousarray(
            xkT.reshape(8, 128, NKC, KC).transpose(2, 1, 0, 3))
        maskb = np.zeros(k_pad, np.float32)
        maskb[ke:] = NEG
        mb_t = _tile_pf(maskb)                       # [128, KT_N]
        KT_N = k_pad // 128
        for g in range(2):
            fs = slice(g * FH, (g + 1) * FH)
            # Wv/bv padded with a ones column per head: the V-projection
            # matmul then produces [V_h | ones] directly (col = 0*x + 1.0).
            Wv_aug = np.zeros((D_MODEL, HPC * 65), np.float32)
            bv_aug = np.zeros(HPC * 65, np.float32)
            for h in range(HPC):
                Wv_aug[:, h * 65:h * 65 + 64] = Wv[:, g * FH + h * 64:
                                                   g * FH + (h + 1) * 64]
                bv_aug[h * 65:h * 65 + 64] = bv[g * FH + h * 64:
                                                g * FH + (h + 1) * 64]
                bv_aug[h * 65 + 64] = 1.0
            in_maps.append({
                "xq": xq,
                "xk": xk,
                "Wq": _tile_pf(np.asarray(Wq[:, fs], np.float32)),
                "Wk": _tile_pf(np.asarray(Wk[:, fs], np.float32)),
                "Wv": _tile_pf(Wv_aug),
                "Wo": _tile_pf(np.asarray(Wo[fs, :], np.float32)),
                "bcst": np.concatenate(
                    [_tile_pf(np.asarray(bq[fs], np.float32)),
                     _tile_pf(np.asarray(bk[fs], np.float32)),
                     mb_t], axis=1).astype(np.float32),
                "bv": bv_aug,
                "consts": consts,
                "zpad": np.zeros(4 * k_pad, np.float32),
                "ident": ident,
            })
    return in_maps, k_pad


def kernel(x, attention_mask, Wq, bq, Wk, bk, Wv, bv, Wo, bo):
    global last_results
    from concourse.bass_utils import run_bass_kernel_spmd

    in_maps, k_pad = _prep_core_inputs(x, attention_mask, Wq, bq, Wk, bk, Wv, bv, Wo)
    nc = _get_compiled(k_pad)
    res = run_bass_kernel_spmd(nc, in_maps, core_ids=list(range(N_CORES)))
    last_results = res

    bo = np.asarray(bo, np.float32)
    out = np.empty((BATCH, SEQ, D_MODEL), np.float32)
    for b in range(BATCH):
        out[b] = res.results[2 * b]["out"] + res.results[2 * b + 1]["out"] + bo
    return out


# revision 13
# speedup vs baseline: 1.7352x; 1.1721x over previous
"""Multi-head attention kernel for 8 Trainium2 NeuronCores.

Problem: B=4, S=2048, D=1024, H=16, Dh=64 MHA with key-side boolean mask.

Sharding: core c handles (batch b = c//2, head-half g = c%2, 8 heads each).
QKV are column-parallel, the output projection is row-parallel (Megatron
style); the host sums the two partial output projections per batch and adds
the output bias.

Host-side preprocessing (pure data marshalling, exact):
  - All inputs are pre-tiled into DMA-native layouts (partition-major,
    contiguous per partition) so each dma_start lowers to large linear
    descriptors instead of thousands of 2KB strided reads.
  - x is transposed per batch (the PE contracts over the partition dim, so
    x^T is required for every projection).
  - Keys with mask=False contribute exactly zero after softmax, so the host
    gathers only the unmasked keys (padded to a multiple of 384 with zero
    rows whose exp-bias is -1e30 => exp == 0 exactly). Exact, and cuts
    score/exp/attn-V work roughly in half.

On-core dataflow:
  xT --(Wk,Wv)--> KT[f,k] (zero-padded per head to K=128), V[k,f] (+biases)
  xT --(Wq)--> QT[f,q]
  scores^T[k,q] = [KT_h ; 0]^T x QT    (fp32r, K=128 full array; zero rows
                                        kill the other head's features)
  E = exp(scores*0.125 + maskbias[k])  (ScalarE, mask fused, bf16 out)
  av[q,65]  = E_tile^T x [V_h | ones]  (bf16 65-col matmuls; col 64 = den)
  avSB[q,f] = av[:,0:64] * (1/den)     (DVE: recip[128,8] + tensor_scalar,
                                        normalization fused into the
                                        PSUM->SBUF copy, bf16 out)
  attnT[f,q] = PE transpose(avSB)      (bf16 transposes per 128x128 block)
  out[s,D] = attnT^T x Wo              (bf16; partial - host adds pair + bo)
"""

import os
import numpy as np

os.environ.setdefault("MYCRO_LOCAL_CACHE", "1")

D_MODEL = 1024
N_HEADS = 16
D_HEAD = 64
BATCH = 4
SEQ = 2048
N_CORES = 8
FH = 512          # features per core (8 heads x 64)
HPC = 8           # heads per core
NEG = -1.0e30     # additive bias for padded/masked keys; exp -> 0 exactly

_COMPILED = {}    # k_pad -> nc
last_results = None  # BassKernelResults of the most recent run (for test.py)


def _build(k_pad):
    """Emit + compile the per-core bass kernel for a given padded key count."""
    import concourse.bacc as bacc
    import concourse.bass as bass
    import concourse.tile as tile
    from concourse import mybir

    f32 = mybir.dt.float32
    f32r = mybir.dt.float32r
    bf16 = mybir.dt.bfloat16
    KT_N = k_pad // 128                     # number of 128-key tiles
    KC = 512 if k_pad % 512 == 0 else 384   # key-side chunk (fp32r needs N>=256)
    assert k_pad % KC == 0 and KC % 128 == 0
    NKC = k_pad // KC

    nc = bacc.Bacc("TRN2", target_bir_lowering=False, debug=False,
                   num_devices=N_CORES)

    # all pre-tiled on host into DMA-native layouts
    dxq = nc.dram_tensor("xq", [4, 128, 8, 512], f32r, kind="ExternalInput")
    dxk = nc.dram_tensor("xk", [NKC, 128, 8, KC], f32r, kind="ExternalInput")
    dWq = nc.dram_tensor("Wq", [128, 8, FH], f32r, kind="ExternalInput")
    dWk = nc.dram_tensor("Wk", [128, 8, FH], f32r, kind="ExternalInput")
    dWv = nc.dram_tensor("Wv", [128, 8, HPC * 65], f32r, kind="ExternalInput")
    dWo = nc.dram_tensor("Wo", [128, 4, D_MODEL], f32r, kind="ExternalInput")
    dbc = nc.dram_tensor("bcst", [128, 8 + KT_N], f32, kind="ExternalInput")
    dbv = nc.dram_tensor("bv", [HPC * 65], f32r, kind="ExternalInput")
    dcst = nc.dram_tensor("consts", [256], f32r, kind="ExternalInput")  # ones|zeros
    dzp = nc.dram_tensor("zpad", [4 * k_pad], f32r, kind="ExternalInput")
    dident = nc.dram_tensor("ident", [128, 128], f32, kind="ExternalInput")
    dout = nc.dram_tensor("out", [SEQ, D_MODEL], f32, kind="ExternalOutput")

    EXP = mybir.ActivationFunctionType.Exp

    from collections import deque

    with tile.TileContext(nc) as tc:
        with tc.tile_pool(name="persist", bufs=1) as pers:
            # ---- pools (stack order: wtq > wof > wtk > xk) ----
            wtq_cm = tc.tile_pool(name="wtq", bufs=1)
            wtq = wtq_cm.__enter__()
            wof_cm = tc.tile_pool(name="wof", bufs=1)
            wofp = wof_cm.__enter__()
            ppool_cm = tc.tile_pool(name="pp", bufs=4, space="PSUM")
            ppool = ppool_cm.__enter__()
            wtk_cm = tc.tile_pool(name="wtk", bufs=1)
            wtk = wtk_cm.__enter__()
            xkp_cm = tc.tile_pool(name="xk", bufs=2)
            xkp = xkp_cm.__enter__()

            # ---- DMAs the first matmuls need come first (chunked per dt) ----
            wk = wtk.tile([128, 8, FH], f32r, tag="wk")
            xk0 = xkp.tile([128, 8, KC], f32r, tag="xk")
            for dt in range(8):
                nc.sync.dma_start(out=wk[:, dt, :], in_=dWk.ap()[:, dt, :])
                nc.sync.dma_start(out=xk0[:, dt, :], in_=dxk.ap()[0][:, dt, :])
            wv = wtk.tile([128, 8, HPC * 65], f32r, tag="wv")
            nc.sync.dma_start(out=wv, in_=dWv.ap())

            # ---- constants in SBUF ----
            bc = pers.tile([128, 8 + KT_N], f32, tag="bcst")
            nc.sync.dma_start(out=bc, in_=dbc.ap())
            bq = bc[:, 0:4]
            bk = bc[:, 4:8]
            mb = bc[:, 8:8 + KT_N]
            bv_row = pers.tile([1, HPC * 65], f32r, tag="bvr")
            nc.sync.dma_start(out=bv_row, in_=dbv.ap()[None, :])
            ones_t = pers.tile([1, 128], f32r, tag="ones")
            nc.sync.dma_start(out=ones_t, in_=dcst.ap()[None, 0:128])
            ones128 = ones_t[:, :]
            ident_f = pers.tile([128, 128], f32, tag="identf")
            nc.sync.dma_start(out=ident_f, in_=dident.ap())
            ident = pers.tile([128, 128], bf16, tag="ident")
            nc.scalar.copy(ident, ident_f)

            # ---- persistent activations ----
            QT = pers.tile([128, 4, SEQ], f32r, tag="QT")        # [f, q]
            # zero-padded per-head score weights: KTe rows 0:64 = even head,
            # rows 64:128 = 0; KTo rows 0:64 = 0, rows 64:128 = odd head.
            KTe = pers.tile([128, 4, k_pad], f32r, tag="KTe")
            KTo = pers.tile([128, 4, k_pad], f32r, tag="KTo")
            Vau = pers.tile([128, KT_N, HPC, 65], bf16, tag="Vau")
            attnT = pers.tile([128, 4, SEQ], bf16, tag="attnT")  # [f, q]
            wo = pers.tile([128, 4, D_MODEL], bf16, tag="wo")

            zin = bass.AP(tensor=dzp.ap().tensor, offset=0,
                          ap=[[0, 64], [1, 4 * k_pad]])
            nc.sync.dma_start(
                out=KTo[0:64, :, :].rearrange("p a k -> p (a k)"), in_=zin)

            wq = wtq.tile([128, 8, FH], f32r, tag="wq")
            nc.sync.dma_start(out=wq, in_=dWq.ap())
            wo_f = wofp.tile([128, 4, D_MODEL], f32r, tag="wof")
            nc.sync.dma_start(out=wo_f, in_=dWo.ap())

            # ================= K/V projections =================
            pk = ppool
            for kc in range(NKC):
                if kc == 0:
                    xk_t = xk0
                else:
                    xk_t = xkp.tile([128, 8, KC], f32r, tag="xk")
                    nc.sync.dma_start(out=xk_t, in_=dxk.ap()[kc])
                for ft in range(4):
                    ps = pk.tile([128, KC], f32, tag="pk")
                    for dt in range(8):
                        nc.tensor.matmul(
                            ps,
                            lhsT=wk[:, dt, ft * 128:(ft + 1) * 128],
                            rhs=xk_t[:, dt, :],
                            start=(dt == 0), stop=(dt == 7))
                    ks = slice(kc * KC, (kc + 1) * KC)
                    nc.vector.tensor_scalar_add(KTe[:, ft, ks], ps,
                                                bk[:, ft:ft + 1])
                for kb in range(KC // 128):
                    kg = kc * (KC // 128) + kb
                    ps = pk.tile([128, HPC * 65], f32, tag="pk")
                    for dt in range(8):
                        nc.tensor.matmul(
                            ps[:, 0:512],
                            lhsT=xk_t[:, dt, kb * 128:(kb + 1) * 128],
                            rhs=wv[:, dt, 0:512],
                            start=(dt == 0), stop=False)
                        nc.tensor.matmul(
                            ps[:, 512:520],
                            lhsT=xk_t[:, dt, kb * 128:(kb + 1) * 128],
                            rhs=wv[:, dt, 512:520],
                            start=(dt == 0), stop=False)
                    nc.tensor.matmul(ps[:, 0:512], lhsT=ones128,
                                     rhs=bv_row[:, 0:512],
                                     start=False, stop=True)
                    nc.tensor.matmul(ps[:, 512:520], lhsT=ones128,
                                     rhs=bv_row[:, 512:520],
                                     start=False, stop=True)
                    nc.scalar.copy(Vau[:, kg, :, :], ps)

            xkp_cm.__exit__(None, None, None)
            wtk_cm.__exit__(None, None, None)

            # finish zero-padded score weights: KTo upper half is a copy of
            # the drained KTe upper half; then KTe upper half becomes zero.
            nc.vector.tensor_copy(
                KTo[64:128, :, :].rearrange("p a k -> p (a k)"),
                KTe[64:128, :, :].rearrange("p a k -> p (a k)"))
            nc.sync.dma_start(
                out=KTe[64:128, :, :].rearrange("p a k -> p (a k)"), in_=zin)

            # convert Wo to bf16 while the Act engine is idle
            nc.scalar.copy(wo.rearrange("p a d -> p (a d)"),
                           wo_f.rearrange("p a d -> p (a d)"))

            ppool_cm.__exit__(None, None, None)
            wof_cm.__exit__(None, None, None)

            # ========== attention + pipelined Q/output projections ==========
            # ScalarE is saturated by the 144 exp tiles; everything else
            # (Q-projection, output projection, normalize, transposes) is
            # scheduled underneath it.  Q/out projection fillers run in a
            # dedicated 1-bank PSUM pool, pumped into the PE stream between
            # attention matmuls.  PSUM banks: s 2x2 + av 2x1 + tp 1 + fill 1.
            with tc.tile_pool(name="et", bufs=3) as etp, \
                 tc.tile_pool(name="asb", bufs=2) as asbp, \
                 tc.tile_pool(name="rp", bufs=4) as rpp, \
                 tc.tile_pool(name="ot", bufs=2) as otp, \
                 tc.tile_pool(name="sp", bufs=2, space="PSUM") as sp, \
                 tc.tile_pool(name="avp", bufs=2, space="PSUM") as avp, \
                 tc.tile_pool(name="tpp", bufs=1, space="PSUM") as tpp, \
                 tc.tile_pool(name="fp", bufs=1, space="PSUM") as fpp, \
                 tc.tile_pool(name="xq", bufs=4) as xqp:

                xq_tiles = []
                for qc in range(4):
                    xq_t = xqp.tile([128, 8, 512], f32r, tag="xq")
                    nc.sync.dma_start(out=xq_t, in_=dxq.ap()[qc])
                    xq_tiles.append(xq_t)

                def qproj_chunk(ps_ap, ft, qc):
                    for dt in range(8):
                        nc.tensor.matmul(
                            ps_ap,
                            lhsT=wq[:, dt, ft * 128:(ft + 1) * 128],
                            rhs=xq_tiles[qc][:, dt, :],
                            start=(dt == 0), stop=(dt == 7))
                        yield
                    nc.vector.tensor_scalar_add(
                        QT[:, ft, qc * 512:(qc + 1) * 512], ps_ap,
                        bq[:, ft:ft + 1])

                def qproj_fill(ft, qc):
                    ps = fpp.tile([128, 512], f32, tag="f")
                    yield from qproj_chunk(ps, ft, qc)

                def outproj_fill(st):
                    ot = otp.tile([128, D_MODEL], f32, tag="ot")
                    for dh in range(2):
                        ps = fpp.tile([128, 512], f32, tag="f")
                        for ft in range(4):
                            nc.tensor.matmul(
                                ps,
                                lhsT=attnT[:, ft, st * 128:(st + 1) * 128],
                                rhs=wo[:, ft, dh * 512:(dh + 1) * 512],
                                start=(ft == 0), stop=(ft == 3))
                            yield
                        nc.vector.tensor_copy(
                            ot[:, dh * 512:(dh + 1) * 512], ps)
                        yield
                    nc.sync.dma_start(
                        out=dout.ap()[st * 128:(st + 1) * 128, :], in_=ot)

                fill = deque()

                def pump(n):
                    for _ in range(n):
                        while fill:
                            try:
                                next(fill[0])
                                break
                            except StopIteration:
                                fill.popleft()
                        else:
                            return

                # QT[:, 0, :] upfront (t=0 scores need it), densely
                # pipelined through the score slots; the rest are fillers,
                # ordered so each block's QT columns land before use.
                for qc in range(4):
                    ps = sp.tile([128, 1024], f32, tag="s")
                    for _ in qproj_chunk(ps[:, 0:512], 0, qc):
                        pass
                for qc2 in range(2):
                    for ft in range(1, 4):
                        for half in range(2):
                            fill.append(qproj_fill(ft, 2 * qc2 + half))

                pending = []   # (avSB, t, q0) transposes not yet emitted

                def flush_pending():
                    for avSB_p, t_p, q0_p in pending:
                        for q8 in range(8):
                            tp = tpp.tile([128, 128], bf16, tag="tp")
                            nc.tensor.transpose(tp, avSB_p[:, q8, :], ident)
                            qs = q0_p + q8 * 128
                            nc.vector.tensor_copy(
                                attnT[:, t_p, qs:qs + 128], tp)
                    pending.clear()

                for qh in range(2):         # query half (1024 queries)
                    for t in range(4):      # head pair (heads 2t, 2t+1)
                        q0 = qh * 1024
                        avSB = asbp.tile([128, 8, 128], bf16, tag="avSB")
                        for F in range(2):  # head 2t+F
                            KT = KTe if F == 0 else KTo
                            h = 2 * t + F

                            def scores(kt):
                                s = sp.tile([128, 1024], f32, tag="s")
                                for hh in range(2):
                                    c0 = q0 + hh * 512
                                    nc.tensor.matmul(
                                        s[:, hh * 512:(hh + 1) * 512],
                                        lhsT=KT[:, t, kt * 128:(kt + 1) * 128],
                                        rhs=QT[:, t, c0:c0 + 512],
                                        start=True, stop=True)
                                return s

                            s_cur = scores(0)
                            if F == 0:
                                flush_pending()
                                if qh == 1 and t == 0:
                                    for st in range(8):
                                        fill.append(outproj_fill(st))
                            # av in two 1-bank tiles (an accumulation group
                            # must not straddle a PSUM bank boundary)
                            av0 = avp.tile([128, 4, 65], f32, tag="av")
                            av1 = avp.tile([128, 4, 65], f32, tag="av")
                            avs = (av0, av1)
                            for kt in range(KT_N):
                                e = etp.tile([128, 1024], bf16, tag="et")
                                nc.scalar.activation(e, s_cur, EXP,
                                                     bias=mb[:, kt:kt + 1],
                                                     scale=0.125)
                                if kt + 1 < KT_N:
                                    s_cur = scores(kt + 1)
                                # PSUM start resets the whole bank: only the
                                # first matmul touching each bank may set it
                                for q8 in (0, 4, 1, 5, 2, 6, 3, 7):
                                    nc.tensor.matmul(
                                        avs[q8 // 4][:, q8 % 4, :],
                                        lhsT=e[:, q8 * 128:(q8 + 1) * 128],
                                        rhs=Vau[:, kt, h, :],
                                        start=(kt == 0 and q8 % 4 == 0),
                                        stop=(kt == KT_N - 1))
                                pump(2)
                            # tail: normalize into avSB cols F*64:(F+1)*64
                            r = rpp.tile([128, 8], f32, tag="r")
                            nc.vector.reciprocal(r[:, 0:4], av0[:, :, 64])
                            nc.vector.reciprocal(r[:, 4:8], av1[:, :, 64])
                            for q8 in range(8):
                                nc.vector.tensor_scalar_mul(
                                    avSB[:, q8, F * 64:(F + 1) * 64],
                                    avs[q8 // 4][:, q8 % 4, 0:64],
                                    r[:, q8:q8 + 1])
                            pump(2)
                        pending.append((avSB, t, q0))

                flush_pending()
                # drain remaining fillers, then the dense output tail
                pump(1 << 20)
                for st in range(8, 16):
                    ps = sp.tile([128, 1024], f32, tag="s")
                    for ft in range(4):
                        for dh in range(2):
                            nc.tensor.matmul(
                                ps[:, dh * 512:(dh + 1) * 512],
                                lhsT=attnT[:, ft, st * 128:(st + 1) * 128],
                                rhs=wo[:, ft, dh * 512:(dh + 1) * 512],
                                start=(ft == 0), stop=(ft == 3))
                    ot = otp.tile([128, D_MODEL], f32, tag="ot")
                    nc.vector.tensor_copy(ot, ps)
                    nc.sync.dma_start(
                        out=dout.ap()[st * 128:(st + 1) * 128, :], in_=ot)

            wtq_cm.__exit__(None, None, None)

    nc.compile()
    return nc


def _get_compiled(k_pad):
    if k_pad not in _COMPILED:
        _COMPILED[k_pad] = _build(k_pad)
    return _COMPILED[k_pad]


def _tile_pf(a, p=128):
    """[P*t, f...] -> contiguous [p, t, f...] partition-major tiling."""
    t = a.shape[0] // p
    return np.ascontiguousarray(
        a.reshape(t, p, *a.shape[1:]).swapaxes(0, 1))


def _prep_core_inputs(x, attention_mask, Wq, bq, Wk, bk, Wv, bv, Wo):
    """Host-side shard prep. Returns (in_maps, k_pad)."""
    x = np.asarray(x, np.float32)
    mask = np.asarray(attention_mask, bool)
    idxs = [np.nonzero(mask[b])[0] for b in range(BATCH)]
    ke_max = max(1, max(len(i) for i in idxs))
    k_pad = 384 * ((ke_max + 383) // 384)
    if k_pad > SEQ:
        k_pad = SEQ
    KC = 512 if k_pad % 512 == 0 else 384
    NKC = k_pad // KC
    KT_N = k_pad // 128

    consts = np.zeros(256, np.float32)
    consts[0:128] = 1.0
    ident = np.eye(128, dtype=np.float32)

    in_maps = []
    for b in range(BATCH):
        xT = x[b].T                                  # [D, S] view
        # xq: [qc, p, dt, 512]
        xq = np.ascontiguousarray(
            xT.reshape(8, 128, 4, 512).transpose(2, 1, 0, 3))
        idx = idxs[b]
        ke = len(idx)
        if ke > k_pad:
            idx = idx[:k_pad]
            ke = k_pad
        xkT = np.zeros((D_MODEL, k_pad), np.float32)
        xkT[:, :ke] = x[b][idx].T
        # xk: [kc, p, dt, KC]
        xk = np.ascontiguousarray(
            xkT.reshape(8, 128, NKC, KC).transpose(2, 1, 0, 3))
        maskb = np.zeros(k_pad, np.float32)
        maskb[ke:] = NEG
        mb_t = _tile_pf(maskb)                       # [128, KT_N]
        KT_N = k_pad // 128
        for g in range(2):
            fs = slice(g * FH, (g + 1) * FH)
            # Wv/bv padded with a ones column per head: the V-projection
            # matmul then produces [V_h | ones] directly (col = 0*x + 1.0).
            Wv_aug = np.zeros((D_MODEL, HPC * 65), np.float32)
            bv_aug = np.zeros(HPC * 65, np.float32)
            for h in range(HPC):
                Wv_aug[:, h * 65:h * 65 + 64] = Wv[:, g * FH + h * 64:
                                                   g * FH + (h + 1) * 64]
                bv_aug[h * 65:h * 65 + 64] = bv[g * FH + h * 64:
                                                g * FH + (h + 1) * 64]
                bv_aug[h * 65 + 64] = 1.0
            in_maps.append({
                "xq": xq,
                "xk": xk,
                "Wq": _tile_pf(np.asarray(Wq[:, fs], np.float32)),
                "Wk": _tile_pf(np.asarray(Wk[:, fs], np.float32)),
                "Wv": _tile_pf(Wv_aug),
                "Wo": _tile_pf(np.asarray(Wo[fs, :], np.float32)),
                "bcst": np.concatenate(
                    [_tile_pf(np.asarray(bq[fs], np.float32)),
                     _tile_pf(np.asarray(bk[fs], np.float32)),
                     mb_t], axis=1).astype(np.float32),
                "bv": bv_aug,
                "consts": consts,
                "zpad": np.zeros(4 * k_pad, np.float32),
                "ident": ident,
            })
    return in_maps, k_pad


def kernel(x, attention_mask, Wq, bq, Wk, bk, Wv, bv, Wo, bo):
    global last_results
    from concourse.bass_utils import run_bass_kernel_spmd

    in_maps, k_pad = _prep_core_inputs(x, attention_mask, Wq, bq, Wk, bk, Wv, bv, Wo)
    nc = _get_compiled(k_pad)
    res = run_bass_kernel_spmd(nc, in_maps, core_ids=list(range(N_CORES)))
    last_results = res

    bo = np.asarray(bo, np.float32)
    out = np.empty((BATCH, SEQ, D_MODEL), np.float32)
    for b in range(BATCH):
        out[b] = res.results[2 * b]["out"] + res.results[2 * b + 1]["out"] + bo
    return out


# revision 14
# speedup vs baseline: 2.0773x; 1.1972x over previous
"""Multi-head attention kernel for 8 Trainium2 NeuronCores.

Problem: B=4, S=2048, D=1024, H=16, Dh=64 MHA with key-side boolean mask.

Sharding: core c handles (batch b = c//2, head-half g = c%2, 8 heads each).
QKV are column-parallel, the output projection is row-parallel (Megatron
style); the host sums the two partial output projections per batch and adds
the output bias.

Host-side preprocessing (pure data marshalling, exact):
  - All inputs are pre-tiled into DMA-native layouts (partition-major,
    contiguous per partition) so each dma_start lowers to large linear
    descriptors instead of thousands of 2KB strided reads.
  - x is transposed per batch (the PE contracts over the partition dim, so
    x^T is required for every projection).
  - Keys with mask=False contribute exactly zero after softmax, so the host
    gathers only the unmasked keys (padded to a multiple of 384 with zero
    rows whose exp-bias is -1e30 => exp == 0 exactly). Exact, and cuts
    score/exp/attn-V work roughly in half.

On-core dataflow:
  xT --(Wk,Wv)--> KT[f,k] (zero-padded per head to K=128), V[k,f] (+biases)
  xT --(Wq)--> QT[f,q]
  scores^T[k,q] = [KT_h ; 0]^T x QT    (fp32r, K=128 full array; zero rows
                                        kill the other head's features)
  E = exp(scores*0.125 + maskbias[k])  (ScalarE, mask fused, bf16 out)
  av[q,65]  = E_tile^T x [V_h | ones]  (bf16 65-col matmuls; col 64 = den)
  avSB[q,f] = av[:,0:64] * (1/den)     (DVE: recip[128,8] + tensor_scalar,
                                        normalization fused into the
                                        PSUM->SBUF copy, bf16 out)
  attnT[f,q] = PE transpose(avSB)      (bf16 transposes per 128x128 block)
  out[s,D] = attnT^T x Wo              (bf16; partial - host adds pair + bo)
"""

import os
import numpy as np
import ml_dtypes

BF16 = ml_dtypes.bfloat16

os.environ.setdefault("MYCRO_LOCAL_CACHE", "1")

D_MODEL = 1024
N_HEADS = 16
D_HEAD = 64
BATCH = 4
SEQ = 2048
N_CORES = 8
FH = 512          # features per core (8 heads x 64)
HPC = 8           # heads per core
NEG = -1.0e30     # additive bias for padded/masked keys; exp -> 0 exactly

_COMPILED = {}    # k_pad -> nc
last_results = None  # BassKernelResults of the most recent run (for test.py)


def _build(k_pad):
    """Emit + compile the per-core bass kernel for a given padded key count."""
    import concourse.bacc as bacc
    import concourse.bass as bass
    import concourse.tile as tile
    from concourse import mybir

    f32 = mybir.dt.float32
    f32r = mybir.dt.float32r
    bf16 = mybir.dt.bfloat16
    KT_N = k_pad // 128                     # number of 128-key tiles
    KC = 512 if k_pad % 512 == 0 else 384   # key-side chunk (fp32r needs N>=256)
    assert k_pad % KC == 0 and KC % 128 == 0
    NKC = k_pad // KC

    nc = bacc.Bacc("TRN2", target_bir_lowering=False, debug=False,
                   num_devices=N_CORES)

    # all pre-tiled on host into DMA-native layouts
    dxq = nc.dram_tensor("xq", [4, 128, 8, 512], bf16, kind="ExternalInput")
    dxk = nc.dram_tensor("xk", [NKC, 128, 8, KC], bf16, kind="ExternalInput")
    dWq = nc.dram_tensor("Wq", [128, 8, FH], bf16, kind="ExternalInput")
    dWk = nc.dram_tensor("Wk", [128, 8, FH], bf16, kind="ExternalInput")
    dWv = nc.dram_tensor("Wv", [128, 8, HPC * 65], bf16, kind="ExternalInput")
    dWo = nc.dram_tensor("Wo", [128, 4, D_MODEL], bf16, kind="ExternalInput")
    dbc = nc.dram_tensor("bcst", [128, 8 + KT_N], f32, kind="ExternalInput")
    dbv = nc.dram_tensor("bv", [HPC * 65], bf16, kind="ExternalInput")
    dcst = nc.dram_tensor("consts", [256], bf16, kind="ExternalInput")  # ones|zeros
    dzp = nc.dram_tensor("zpad", [4 * k_pad], bf16, kind="ExternalInput")
    dident = nc.dram_tensor("ident", [128, 128], bf16, kind="ExternalInput")
    dout = nc.dram_tensor("out", [SEQ, D_MODEL], f32, kind="ExternalOutput")

    EXP = mybir.ActivationFunctionType.Exp

    from collections import deque

    with tile.TileContext(nc) as tc:
        with tc.tile_pool(name="persist", bufs=1) as pers:
            # ---- pools (stack order: wtq > wof > wtk > xk) ----
            wtq_cm = tc.tile_pool(name="wtq", bufs=1)
            wtq = wtq_cm.__enter__()
            ppool_cm = tc.tile_pool(name="pp", bufs=4, space="PSUM")
            ppool = ppool_cm.__enter__()
            wtk_cm = tc.tile_pool(name="wtk", bufs=1)
            wtk = wtk_cm.__enter__()
            xkp_cm = tc.tile_pool(name="xk", bufs=2)
            xkp = xkp_cm.__enter__()

            # ---- DMAs the first matmuls need come first (chunked per dt) ----
            wk = wtk.tile([128, 8, FH], bf16, tag="wk")
            xk0 = xkp.tile([128, 8, KC], bf16, tag="xk")
            for dt in range(8):
                nc.sync.dma_start(out=wk[:, dt, :], in_=dWk.ap()[:, dt, :])
                nc.sync.dma_start(out=xk0[:, dt, :], in_=dxk.ap()[0][:, dt, :])
            wv = wtk.tile([128, 8, HPC * 65], bf16, tag="wv")
            nc.sync.dma_start(out=wv, in_=dWv.ap())

            # ---- constants in SBUF ----
            bc = pers.tile([128, 8 + KT_N], f32, tag="bcst")
            nc.sync.dma_start(out=bc, in_=dbc.ap())
            bq = bc[:, 0:4]
            bk = bc[:, 4:8]
            mb = bc[:, 8:8 + KT_N]
            bv_row = pers.tile([1, HPC * 65], bf16, tag="bvr")
            nc.sync.dma_start(out=bv_row, in_=dbv.ap()[None, :])
            ones_t = pers.tile([1, 128], bf16, tag="ones")
            nc.sync.dma_start(out=ones_t, in_=dcst.ap()[None, 0:128])
            ones128 = ones_t[:, :]
            ident = pers.tile([128, 128], bf16, tag="ident")
            nc.sync.dma_start(out=ident, in_=dident.ap())

            # ---- persistent activations ----
            QT = pers.tile([128, 4, SEQ], bf16, tag="QT")        # [f, q]
            # zero-padded per-head score weights: KTe rows 0:64 = even head,
            # rows 64:128 = 0; KTo rows 0:64 = 0, rows 64:128 = odd head.
            KTe = pers.tile([128, 4, k_pad], bf16, tag="KTe")
            KTo = pers.tile([128, 4, k_pad], bf16, tag="KTo")
            Vau = pers.tile([128, KT_N, HPC, 65], bf16, tag="Vau")
            attnT = pers.tile([128, 4, SEQ], bf16, tag="attnT")  # [f, q]
            wo = pers.tile([128, 4, D_MODEL], bf16, tag="wo")

            zin = bass.AP(tensor=dzp.ap().tensor, offset=0,
                          ap=[[0, 64], [1, 4 * k_pad]])
            nc.sync.dma_start(
                out=KTo[0:64, :, :].rearrange("p a k -> p (a k)"), in_=zin)

            wq = wtq.tile([128, 8, FH], bf16, tag="wq")
            nc.sync.dma_start(out=wq, in_=dWq.ap())
            nc.sync.dma_start(out=wo.rearrange("p a d -> p (a d)"),
                              in_=dWo.ap().rearrange("p a d -> p (a d)"))

            # ================= K/V projections =================
            pk = ppool
            for kc in range(NKC):
                if kc == 0:
                    xk_t = xk0
                else:
                    xk_t = xkp.tile([128, 8, KC], bf16, tag="xk")
                    nc.sync.dma_start(out=xk_t, in_=dxk.ap()[kc])
                for ft in range(4):
                    ps = pk.tile([128, KC], f32, tag="pk")
                    for dt in range(8):
                        nc.tensor.matmul(
                            ps,
                            lhsT=wk[:, dt, ft * 128:(ft + 1) * 128],
                            rhs=xk_t[:, dt, :],
                            start=(dt == 0), stop=(dt == 7))
                    ks = slice(kc * KC, (kc + 1) * KC)
                    nc.vector.tensor_scalar_add(KTe[:, ft, ks], ps,
                                                bk[:, ft:ft + 1])
                for kb in range(KC // 128):
                    kg = kc * (KC // 128) + kb
                    ps = pk.tile([128, HPC * 65], f32, tag="pk")
                    for dt in range(8):
                        nc.tensor.matmul(
                            ps[:, 0:512],
                            lhsT=xk_t[:, dt, kb * 128:(kb + 1) * 128],
                            rhs=wv[:, dt, 0:512],
                            start=(dt == 0), stop=False)
                        nc.tensor.matmul(
                            ps[:, 512:520],
                            lhsT=xk_t[:, dt, kb * 128:(kb + 1) * 128],
                            rhs=wv[:, dt, 512:520],
                            start=(dt == 0), stop=False)
                    nc.tensor.matmul(ps[:, 0:512], lhsT=ones128,
                                     rhs=bv_row[:, 0:512],
                                     start=False, stop=True)
                    nc.tensor.matmul(ps[:, 512:520], lhsT=ones128,
                                     rhs=bv_row[:, 512:520],
                                     start=False, stop=True)
                    nc.scalar.copy(Vau[:, kg, :, :], ps)

            xkp_cm.__exit__(None, None, None)
            wtk_cm.__exit__(None, None, None)

            # finish zero-padded score weights: KTo upper half is a copy of
            # the drained KTe upper half; then KTe upper half becomes zero.
            nc.vector.tensor_copy(
                KTo[64:128, :, :].rearrange("p a k -> p (a k)"),
                KTe[64:128, :, :].rearrange("p a k -> p (a k)"))
            nc.sync.dma_start(
                out=KTe[64:128, :, :].rearrange("p a k -> p (a k)"), in_=zin)

            ppool_cm.__exit__(None, None, None)

            # ========== attention + pipelined Q/output projections ==========
            # ScalarE is saturated by the 144 exp tiles; everything else
            # (Q-projection, output projection, normalize, transposes) is
            # scheduled underneath it.  Q/out projection fillers run in a
            # dedicated 1-bank PSUM pool, pumped into the PE stream between
            # attention matmuls.  PSUM banks: s 2x2 + av 2x1 + tp 1 + fill 1.
            with tc.tile_pool(name="et", bufs=3) as etp, \
                 tc.tile_pool(name="asb", bufs=2) as asbp, \
                 tc.tile_pool(name="rp", bufs=4) as rpp, \
                 tc.tile_pool(name="ot", bufs=2) as otp, \
                 tc.tile_pool(name="sp", bufs=2, space="PSUM") as sp, \
                 tc.tile_pool(name="avp", bufs=2, space="PSUM") as avp, \
                 tc.tile_pool(name="tpp", bufs=1, space="PSUM") as tpp, \
                 tc.tile_pool(name="fp", bufs=1, space="PSUM") as fpp, \
                 tc.tile_pool(name="xq", bufs=4) as xqp:

                xq_tiles = []
                for qc in range(4):
                    xq_t = xqp.tile([128, 8, 512], bf16, tag="xq")
                    nc.sync.dma_start(out=xq_t, in_=dxq.ap()[qc])
                    xq_tiles.append(xq_t)

                def qproj_chunk(ps_ap, ft, qc):
                    for dt in range(8):
                        nc.tensor.matmul(
                            ps_ap,
                            lhsT=wq[:, dt, ft * 128:(ft + 1) * 128],
                            rhs=xq_tiles[qc][:, dt, :],
                            start=(dt == 0), stop=(dt == 7))
                        yield
                    nc.vector.tensor_scalar_add(
                        QT[:, ft, qc * 512:(qc + 1) * 512], ps_ap,
                        bq[:, ft:ft + 1])

                def qproj_fill(ft, qc):
                    ps = fpp.tile([128, 512], f32, tag="f")
                    yield from qproj_chunk(ps, ft, qc)

                def outproj_fill(st):
                    ot = otp.tile([128, D_MODEL], f32, tag="ot")
                    for dh in range(2):
                        ps = fpp.tile([128, 512], f32, tag="f")
                        for ft in range(4):
                            nc.tensor.matmul(
                                ps,
                                lhsT=attnT[:, ft, st * 128:(st + 1) * 128],
                                rhs=wo[:, ft, dh * 512:(dh + 1) * 512],
                                start=(ft == 0), stop=(ft == 3))
                            yield
                        nc.vector.tensor_copy(
                            ot[:, dh * 512:(dh + 1) * 512], ps)
                        yield
                    nc.sync.dma_start(
                        out=dout.ap()[st * 128:(st + 1) * 128, :], in_=ot)

                fill = deque()

                def pump(n):
                    for _ in range(n):
                        while fill:
                            try:
                                next(fill[0])
                                break
                            except StopIteration:
                                fill.popleft()
                        else:
                            return

                # QT[:, 0, :] upfront (t=0 scores need it), densely
                # pipelined through the score slots; the rest are fillers,
                # ordered so each block's QT columns land before use.
                for qc in range(4):
                    ps = sp.tile([128, 1024], f32, tag="s")
                    for _ in qproj_chunk(ps[:, 0:512], 0, qc):
                        pass
                for qc2 in range(2):
                    for ft in range(1, 4):
                        for half in range(2):
                            fill.append(qproj_fill(ft, 2 * qc2 + half))

                pending = []   # (avSB, t, q0) transposes not yet emitted

                def flush_pending():
                    for avSB_p, t_p, q0_p in pending:
                        for q8 in range(8):
                            tp = tpp.tile([128, 128], bf16, tag="tp")
                            nc.tensor.transpose(tp, avSB_p[:, q8, :], ident)
                            qs = q0_p + q8 * 128
                            nc.vector.tensor_copy(
                                attnT[:, t_p, qs:qs + 128], tp)
                    pending.clear()

                for qh in range(2):         # query half (1024 queries)
                    for t in range(4):      # head pair (heads 2t, 2t+1)
                        q0 = qh * 1024
                        avSB = asbp.tile([128, 8, 128], bf16, tag="avSB")
                        for F in range(2):  # head 2t+F
                            KT = KTe if F == 0 else KTo
                            h = 2 * t + F

                            def scores(kt):
                                s = sp.tile([128, 1024], f32, tag="s")
                                for hh in range(2):
                                    c0 = q0 + hh * 512
                                    nc.tensor.matmul(
                                        s[:, hh * 512:(hh + 1) * 512],
                                        lhsT=KT[:, t, kt * 128:(kt + 1) * 128],
                                        rhs=QT[:, t, c0:c0 + 512],
                                        start=True, stop=True)
                                return s

                            s_cur = scores(0)
                            if F == 0:
                                flush_pending()
                                if qh == 1 and t == 0:
                                    for st in range(8):
                                        fill.append(outproj_fill(st))
                            # av in two 1-bank tiles (an accumulation group
                            # must not straddle a PSUM bank boundary)
                            av0 = avp.tile([128, 4, 65], f32, tag="av")
                            av1 = avp.tile([128, 4, 65], f32, tag="av")
                            avs = (av0, av1)
                            for kt in range(KT_N):
                                e = etp.tile([128, 1024], bf16, tag="et")
                                nc.scalar.activation(e, s_cur, EXP,
                                                     bias=mb[:, kt:kt + 1],
                                                     scale=0.125)
                                if kt + 1 < KT_N:
                                    s_cur = scores(kt + 1)
                                # PSUM start resets the whole bank: only the
                                # first matmul touching each bank may set it
                                for q8 in (0, 4, 1, 5, 2, 6, 3, 7):
                                    nc.tensor.matmul(
                                        avs[q8 // 4][:, q8 % 4, :],
                                        lhsT=e[:, q8 * 128:(q8 + 1) * 128],
                                        rhs=Vau[:, kt, h, :],
                                        start=(kt == 0 and q8 % 4 == 0),
                                        stop=(kt == KT_N - 1))
                                pump(1)
                            # tail: normalize into avSB cols F*64:(F+1)*64
                            r = rpp.tile([128, 8], f32, tag="r")
                            nc.vector.reciprocal(r[:, 0:4], av0[:, :, 64])
                            nc.vector.reciprocal(r[:, 4:8], av1[:, :, 64])
                            for q8 in range(8):
                                nc.vector.tensor_scalar_mul(
                                    avSB[:, q8, F * 64:(F + 1) * 64],
                                    avs[q8 // 4][:, q8 % 4, 0:64],
                                    r[:, q8:q8 + 1])
                            pump(2)
                        pending.append((avSB, t, q0))

                flush_pending()
                # drain remaining fillers, then the dense output tail
                pump(1 << 20)
                for st in range(8, 16):
                    ps = sp.tile([128, 1024], f32, tag="s")
                    for ft in range(4):
                        for dh in range(2):
                            nc.tensor.matmul(
                                ps[:, dh * 512:(dh + 1) * 512],
                                lhsT=attnT[:, ft, st * 128:(st + 1) * 128],
                                rhs=wo[:, ft, dh * 512:(dh + 1) * 512],
                                start=(ft == 0), stop=(ft == 3))
                    ot = otp.tile([128, D_MODEL], f32, tag="ot")
                    if st % 2 == 0:
                        nc.scalar.copy(ot, ps)
                    else:
                        nc.vector.tensor_copy(ot, ps)
                    nc.sync.dma_start(
                        out=dout.ap()[st * 128:(st + 1) * 128, :], in_=ot)

            wtq_cm.__exit__(None, None, None)

    nc.compile()
    return nc


def _get_compiled(k_pad):
    if k_pad not in _COMPILED:
        _COMPILED[k_pad] = _build(k_pad)
    return _COMPILED[k_pad]


def _tile_pf(a, p=128):
    """[P*t, f...] -> contiguous [p, t, f...] partition-major tiling."""
    t = a.shape[0] // p
    return np.ascontiguousarray(
        a.reshape(t, p, *a.shape[1:]).swapaxes(0, 1))


def _prep_core_inputs(x, attention_mask, Wq, bq, Wk, bk, Wv, bv, Wo):
    """Host-side shard prep. Returns (in_maps, k_pad)."""
    x = np.asarray(x, np.float32)
    mask = np.asarray(attention_mask, bool)
    idxs = [np.nonzero(mask[b])[0] for b in range(BATCH)]
    ke_max = max(1, max(len(i) for i in idxs))
    k_pad = 384 * ((ke_max + 383) // 384)
    if k_pad > SEQ:
        k_pad = SEQ
    KC = 512 if k_pad % 512 == 0 else 384
    NKC = k_pad // KC
    KT_N = k_pad // 128

    consts = np.zeros(256, BF16)
    consts[0:128] = 1.0
    ident = np.eye(128, dtype=BF16)

    in_maps = []
    for b in range(BATCH):
        xT = x[b].T                                  # [D, S] view
        # xq: [qc, p, dt, 512]
        xq = np.ascontiguousarray(
            xT.reshape(8, 128, 4, 512).transpose(2, 1, 0, 3)).astype(BF16)
        idx = idxs[b]
        ke = len(idx)
        if ke > k_pad:
            idx = idx[:k_pad]
            ke = k_pad
        xkT = np.zeros((D_MODEL, k_pad), np.float32)
        xkT[:, :ke] = x[b][idx].T
        # xk: [kc, p, dt, KC]
        xk = np.ascontiguousarray(
            xkT.reshape(8, 128, NKC, KC).transpose(2, 1, 0, 3)).astype(BF16)
        maskb = np.zeros(k_pad, np.float32)
        maskb[ke:] = NEG
        mb_t = _tile_pf(maskb)                       # [128, KT_N]
        KT_N = k_pad // 128
        for g in range(2):
            fs = slice(g * FH, (g + 1) * FH)
            # Wv/bv padded with a ones column per head: the V-projection
            # matmul then produces [V_h | ones] directly (col = 0*x + 1.0).
            Wv_aug = np.zeros((D_MODEL, HPC * 65), np.float32)
            bv_aug = np.zeros(HPC * 65, np.float32)
            for h in range(HPC):
                Wv_aug[:, h * 65:h * 65 + 64] = Wv[:, g * FH + h * 64:
                                                   g * FH + (h + 1) * 64]
                bv_aug[h * 65:h * 65 + 64] = bv[g * FH + h * 64:
                                                g * FH + (h + 1) * 64]
                bv_aug[h * 65 + 64] = 1.0
            in_maps.append({
                "xq": xq,
                "xk": xk,
                "Wq": _tile_pf(np.asarray(Wq[:, fs], BF16)),
                "Wk": _tile_pf(np.asarray(Wk[:, fs], BF16)),
                "Wv": _tile_pf(Wv_aug.astype(BF16)),
                "Wo": _tile_pf(np.asarray(Wo[fs, :], BF16)),
                "bcst": np.concatenate(
                    [_tile_pf(np.asarray(bq[fs], np.float32)),
                     _tile_pf(np.asarray(bk[fs], np.float32)),
                     mb_t], axis=1).astype(np.float32),
                "bv": bv_aug.astype(BF16),
                "consts": consts,
                "zpad": np.zeros(4 * k_pad, BF16),
                "ident": ident,
            })
    return in_maps, k_pad


def kernel(x, attention_mask, Wq, bq, Wk, bk, Wv, bv, Wo, bo):
    global last_results
    from concourse.bass_utils import run_bass_kernel_spmd

    in_maps, k_pad = _prep_core_inputs(x, attention_mask, Wq, bq, Wk, bk, Wv, bv, Wo)
    nc = _get_compiled(k_pad)
    res = run_bass_kernel_spmd(nc, in_maps, core_ids=list(range(N_CORES)))
    last_results = res

    bo = np.asarray(bo, np.float32)
    out = np.empty((BATCH, SEQ, D_MODEL), np.float32)
    for b in range(BATCH):
        out[b] = res.results[2 * b]["out"] + res.results[2 * b + 1]["out"] + bo
    return out
